# revision 11
# baseline (speedup 1.0000x reference)
"""Trainium2 Bass kernel for ConvNdFunc: 16x16/stride-8 patch MLP (256->1024->1).

Data-parallel over batch: 32 images -> 8 cores x 4 images, no collectives.

v2 design (vs 172us baseline): the host performs the FULL im2col, so the
device loads exactly the matmul operands (4.1MB/core vs 33MB replicated
rows) and the on-chip GpSimd extract disappears. The second linear layer
is moved entirely off the PE data port:

  - Super-tile = 14 window-rows = 896 windows (padded 64/row), 18 per core.
  - L1 (TensorE data port): per hidden block hb, 4 matmuls of 448 cols
    (2 column halves x 2 K-chunks, chunk 1 = same yb shifted one window
    column). 32 matmuls/super-tile -> PSUM f32 [128, 2x512-bank].
  - ReLU: Scalar activation (blocks 0-4,7) / Vector max (5,6), fp16 out.
  - W2 fold: S += w2[p,hb] * hsw_hb via scalar_tensor_tensor (per-partition
    scalar multiply-accumulate), Vector blocks 0-5, GpSimd 6-7. S is the
    per-partition-weighted hidden sum, [128, 896] fp16.
  - L2 reduction: 7 matmuls per super-tile with stationary = S[:,128c:128c+128]
    (enters the PE via the *weight port*, which runs parallel to the data
    port) and moving = ones[128,1] (1 data-port cycle). Interleaved into
    the L1 stream of super-tile s+2 so LDWEIGHTS hides under 448-col MMs.
  - Output: ops[128,7] PSUM -> +b2 -> two small scatter DMAs per super-tile
    (partition p, chunk c) -> y rows 14s+2c+p//64, col p%64.

fp16 is used on the whole 16-bit path (x, W1, hsw, S): same PE/DVE speed
as bf16, 3 extra mantissa bits.
"""

import os
import sys
from contextlib import ExitStack

_RT = "/opt/trn_rl_repo"
if _RT not in sys.path:
    sys.path.insert(0, _RT)

import numpy as np


def _ensure_ntff_hook():
    """Register the axon NTFF profiling hook if the image's antenv lacks it.

    Only matters when tracing (KERNEL_TRACE=1); no-op side effects otherwise.
    """
    import types

    try:
        import antenv.axon_hooks  # noqa: F401

        return
    except ImportError:
        pass
    try:
        import antenv
    except ImportError:
        return
    mod = types.ModuleType("antenv.axon_hooks")
    _state = {"hook": None}
    mod.set_axon_ntff_profile_hook = lambda h: _state.__setitem__("hook", h)
    mod.get_axon_ntff_profile_hook = lambda: _state["hook"]
    sys.modules["antenv.axon_hooks"] = mod
    antenv.axon_hooks = mod
    try:
        from trn_agent_boot.trn_boot import _ntff_profile_via_ctypes

        mod.set_axon_ntff_profile_hook(
            _ntff_profile_via_ctypes("/opt/axon/libaxon_pjrt.so")
        )
    except Exception:
        pass


_ensure_ntff_hook()

import concourse.bass as bass
import concourse.tile as tile
from concourse import bacc, mybir
from concourse.bass_utils import run_bass_kernel_spmd

B, H, W = 32, 512, 512
KK, S, HID = 16, 8, 1024
OH = OW = (H - KK) // S + 1  # 63
NCORES = 8
BPC = B // NCORES  # 4 images per core
WPAD = W + S  # pad columns so kw-phase-shifted reads stay in bounds
ROWS = BPC * OH  # 252 window-rows per core (contiguous across images)
ST = 14  # window-rows per super-tile
NST = ROWS // ST  # 18 super-tiles
OWP = OW + 1  # padded window columns per row (col 63 discarded)
NWP = ST * OWP  # 896 windows per super-tile
YBC = NWP + S  # 904 yb columns (8 slop cols, only col 896 read -> pad)
NHB = HID // 128  # 8 hidden blocks
NCH = NWP // 128  # 7 L2 chunks of 128 windows

F16 = np.float16
F32 = mybir.dt.float32
F16_T = mybir.dt.float16

LAST_RESULTS = None  # BassKernelResults of the most recent run (for test harness)


def _build_nc(b2_val: float, b1_nonzero: bool):
    nc = bacc.Bacc(None, target_bir_lowering=False)

    # host im2col: x_d[s, p, i*64+j] = x[row 14s+i -> image/patch row, 8j + p%8]
    # with p = kh*8 + kwp; patch row = 8*((14s+i)%63) + kh of image (14s+i)//63.
    x_d = nc.dram_tensor("x", [NST, 128, YBC], F16_T, kind="ExternalInput")
    w1_d = nc.dram_tensor("w1", [128, 2, HID], F16_T, kind="ExternalInput")
    w2_d = nc.dram_tensor("w2", [128, NHB], F32, kind="ExternalInput")
    b1_d = nc.dram_tensor("b1", [128, NHB], F32, kind="ExternalInput")
    y_d = nc.dram_tensor("y", [ROWS, OW], F32, kind="ExternalOutput")

    relu = mybir.ActivationFunctionType.Relu
    y_flat = y_d[:, :]

    # relu engine per hidden block: 's'=ScalarE activation, 'v'=VectorE max
    # (scalar_tensor_tensor is rejected by codegen on the Pool/GpSimd engine,
    # so the whole w2-scale-accumulate chain lives on VectorE)
    relu_eng = ["s", "s", "s", "s", "s", "v", "v", "s"]

    with tile.TileContext(nc) as tc, ExitStack() as ctx:
        consts = ctx.enter_context(tc.tile_pool(name="consts", bufs=1))
        yb_pool = ctx.enter_context(tc.tile_pool(name="yb", bufs=3))
        hsw_pool = ctx.enter_context(tc.tile_pool(name="hsw", bufs=10))
        s_pool = ctx.enter_context(tc.tile_pool(name="spool", bufs=3))
        osb_pool = ctx.enter_context(tc.tile_pool(name="osb", bufs=2))
        ht_pool = ctx.enter_context(tc.tile_pool(name="ht", bufs=3, space="PSUM"))
        ops_pool = ctx.enter_context(tc.tile_pool(name="ops", bufs=2, space="PSUM"))

        w1_sb = consts.tile([128, 2, HID], F16_T)
        w2_sb = consts.tile([128, NHB], F32)
        ones_sb = consts.tile([128, 1], F16_T)
        nc.vector.memset(ones_sb, 1.0)

        # PE is data-blocked for the first ~3.4us; run zero matmuls meanwhile
        # so the HAM clock gate is already at 2.4 GHz when real work arrives.
        warm_in = consts.tile([128, 512], F16_T)
        nc.vector.memset(warm_in, 0.0)
        warm_ps = ht_pool.tile([128, 1024], F32, tag="ht")
        for _ in range(18):
            nc.tensor.matmul(
                warm_ps[:, 0:512], warm_in[:, 0:128], warm_in, start=True, stop=True
            )

        if b1_nonzero:
            b1_sb = consts.tile([128, NHB], F32)
            nc.scalar.dma_start(out=b1_sb, in_=b1_d[:, :])

        # pending L2 reductions: list of (st_index, S_tile, ops_tile, next_chunk)
        pending = []
        done_l2 = []  # (st_index, ops_tile) ready for output

        def emit_l2_chunk(cur_s):
            # only reduce super-tile s-2: S(t) finishes ~5us after L1(t), so
            # an earlier emit would head-of-line-stall the PE queue
            if not pending or pending[0][0] > cur_s - 2:
                return
            t, s_t, ops_t, c = pending[0]
            nc.tensor.matmul(
                ops_t[:, c : c + 1],
                s_t[:, 128 * c : 128 * (c + 1)],
                ones_sb,
                start=True,
                stop=True,
            )
            if c + 1 == NCH:
                pending.pop(0)
                done_l2.append((t, ops_t))
            else:
                pending[0] = (t, s_t, ops_t, c + 1)

        def emit_output():
            t, ops_t = done_l2.pop(0)
            osb = osb_pool.tile([128, NHB], F32)
            nc.scalar.add(osb[:, 0:NCH], ops_t[:, 0:NCH], b2_val)
            for par in range(2):
                src = osb[64 * par : 64 * par + 63, 0:NCH]
                dst = bass.AP(
                    tensor=y_flat.tensor,
                    offset=(ST * t + par) * OW,
                    ap=[[1, OW], [2 * OW, NCH]],
                )
                nc.scalar.dma_start(out=dst, in_=src)

        for s in range(NST):
            yb = yb_pool.tile([128, YBC], F16_T)
            if s == 0:
                # split first loads across both HWDGE rings to cut cold-start
                # latency; weights emitted after so tile-0 data isn't queued
                # behind them
                nc.sync.dma_start(out=yb[:, 0:452], in_=x_d[s, :, 0:452])
                nc.scalar.dma_start(out=yb[:, 452:YBC], in_=x_d[s, :, 452:YBC])
                nc.sync.dma_start(out=w1_sb[:, 0, :], in_=w1_d[:, 0, :])
                nc.scalar.dma_start(out=w1_sb[:, 1, :], in_=w1_d[:, 1, :])
                nc.scalar.dma_start(out=w2_sb, in_=w2_d[:, :])
            else:
                nc.sync.dma_start(out=yb, in_=x_d[s])

            hsw_tiles = []
            mm = 0
            for hb in range(NHB):
                ht = ht_pool.tile([128, 1024], F32)
                for h in range(2):
                    for c in range(2):
                        nc.tensor.matmul(
                            ht[:, 512 * h : 512 * h + 448],
                            w1_sb[:, c, hb * 128 : (hb + 1) * 128],
                            yb[:, 448 * h + c : 448 * h + c + 448],
                            start=(c == 0),
                            stop=(c == 1),
                        )
                        mm += 1
                        # interleave one L2 chunk of super-tile s-2 per L1
                        # matmul once its PSUM deps are pipelined (mm >= 9)
                        if mm >= 9 and (mm - 9) % 3 == 0:
                            emit_l2_chunk(s)
                # ReLU block hb -> fp16 hsw (dense [128, 896])
                hsw = hsw_pool.tile([128, NWP], F16_T)
                hsw_tiles.append(hsw)
                ht_ap = bass.AP(
                    tensor=ht.tensor,
                    offset=ht.offset,
                    ap=[ht.ap[0], [512, 2], [1, 448]],
                )
                hsw_ap = bass.AP(
                    tensor=hsw.tensor,
                    offset=hsw.offset,
                    ap=[hsw.ap[0], [448, 2], [1, 448]],
                )
                if relu_eng[hb] == "s":
                    nc.scalar.activation(
                        out=hsw_ap,
                        in_=ht_ap,
                        func=relu,
                        bias=b1_sb[:, hb : hb + 1] if b1_nonzero else 0.0,
                    )
                else:
                    if b1_nonzero:
                        nc.vector.tensor_scalar(
                            hsw_ap,
                            ht_ap,
                            b1_sb[:, hb : hb + 1],
                            0.0,
                            op0=mybir.AluOpType.add,
                            op1=mybir.AluOpType.max,
                        )
                    else:
                        nc.vector.tensor_scalar_max(hsw_ap, ht_ap, 0.0)

            # S accumulation: S = sum_hb w2[:,hb] * hsw_hb  (fp16, SBUF)
            s_t = s_pool.tile([128, NWP], F16_T)
            for hb in range(NHB):
                if hb == 0:
                    nc.vector.tensor_scalar_mul(
                        s_t[:, :], hsw_tiles[0][:, :], w2_sb[:, 0:1]
                    )
                else:
                    nc.vector.scalar_tensor_tensor(
                        out=s_t[:, :],
                        in0=hsw_tiles[hb][:, :],
                        scalar=w2_sb[:, hb : hb + 1],
                        in1=s_t[:, :],
                        op0=mybir.AluOpType.mult,
                        op1=mybir.AluOpType.add,
                    )

            ops_t = ops_pool.tile([128, NHB], F32)
            pending.append((s, s_t, ops_t, 0))
            if done_l2:
                emit_output()

        # tail: drain remaining L2 reductions + outputs
        while pending:
            emit_l2_chunk(NST + 2)
            while done_l2:
                emit_output()

    nc.finalize()
    return nc


def kernel(x, W1, b1, W2, b2):
    global LAST_RESULTS
    x = np.asarray(x, dtype=np.float32)
    W1 = np.asarray(W1, dtype=np.float32)
    b1 = np.asarray(b1, dtype=np.float32)
    W2 = np.asarray(W2, dtype=np.float32)
    b2 = np.asarray(b2, dtype=np.float32)

    xp = np.zeros((B, H, WPAD), dtype=F16)
    xp[:, :, :W] = x.astype(F16)
    # full im2col on the host: wr[b, wrow, kh, kwp, j] = xp[b, 8*wrow+kh, 8*j+kwp]
    sb, sr, se = xp.strides
    wr = np.lib.stride_tricks.as_strided(
        xp, shape=(B, OH, KK, S, OWP), strides=(sb, 8 * sr, sr, se, 8 * se)
    )
    wrm = np.ascontiguousarray(wr).reshape(B * OH, 128, OWP)

    # W1 row r = kh*16 + kw; chunk c, partition p=(kh*8+kwp) <- row kh*16+8c+kwp
    w1p = (
        W1.reshape(KK, 2, S, HID).transpose(0, 2, 1, 3).reshape(128, 2, HID)
    ).astype(F16)
    w2p = np.ascontiguousarray(W2.reshape(NHB, 128).T)  # [p, hb] f32
    b1p = np.ascontiguousarray(b1.reshape(NHB, 128).T)  # [p, hb] f32
    b1_nonzero = bool(np.any(b1 != 0.0))
    b2_val = float(b2.reshape(-1)[0])

    nc = _build_nc(b2_val, b1_nonzero)

    in_maps = []
    for core in range(NCORES):
        a = wrm[core * ROWS : (core + 1) * ROWS]  # [252, 128, 64]
        yv = np.zeros((NST, 128, YBC), dtype=F16)
        yv[:, :, :NWP] = (
            a.reshape(NST, ST, 128, OWP).transpose(0, 2, 1, 3).reshape(NST, 128, NWP)
        )
        in_maps.append(
            {
                "x": np.ascontiguousarray(yv),
                "w1": w1p,
                "w2": w2p,
                "b1": b1p,
            }
        )

    LAST_RESULTS = run_bass_kernel_spmd(
        nc,
        in_maps,
        core_ids=list(range(NCORES)),
        trace=bool(int(os.environ.get("KERNEL_TRACE", "0") or "0")),
    )
    y = np.concatenate([r["y"] for r in LAST_RESULTS.results], axis=0)
    return y.reshape(B, OH, OW).astype(np.float32)
